# revision 1
# baseline (speedup 1.0000x reference)
"""MixLinear int4-GEMM kernel for 8x TRN2 NeuronCores.

Strategy: tensor-parallel over out_features (each core owns OUT/8 = 512
output channels; q_weight / scale_col / weight_cache are sharded along the
output dim; x is replicated).  Per core:

  1. Per 128-row activation tile: masked abs-max (outlier cols excluded) in
     one fused DVE tensor_tensor_reduce pass; x_scale = max/7, r = 1/x_scale.
  2. Magic-number RNE round: t = xz*r + 1.5*2^23 on ScalarE, q = t - magic on
     GPSIMD (exact small ints, cast to bf16).
  3. q is transposed to contraction-major layout with one DMA-xbar transpose.
  4. int4 weights are unpacked on-device (DVE bitwise ops on the packed
     bytes) into a [K, 32, 512] bf16 wT resident in SBUF, transposed by
     DMA-xbar.
  5. 32 bf16 matmuls (exact: integer values) + 2 outlier matmuls accumulate
     into one PSUM bank.  The outlier operands are pre-scaled by 1/x_scale
     (per row) and 1/scale_col (per out channel) so one dequant applies to
     the whole PSUM: y = psum * x_scale * scale_col, fused into the PSUM
     eviction (ScalarE per-partition scale, DVE broadcast multiply).

The output shard [8192, 512] is DMA'd out; the host concatenates shards.
"""

import numpy as np

B, S, IN, OUT, FP = 4, 2048, 4096, 4096, 256
M = B * S
NCORES = 8
OS = OUT // NCORES  # out-features shard per core
QMAX = 7.0
MAGIC = 12582912.0  # 1.5 * 2**23: adding+subtracting forces RNE to integer


def emit_core_kernel(nc, tc, m, in_dim, os_dim, fp_dim):
    """Emit the per-core tile program. All dims compile-time constants."""
    import os as _os

    import concourse.bass as bass
    import concourse.mybir as mybir
    from concourse.masks import make_identity

    DBG = set(_os.environ.get("KERNEL_DISABLE", "").split(","))

    f32 = mybir.dt.float32
    f32r = mybir.dt.float32r
    bf16 = mybir.dt.bfloat16
    i32 = mybir.dt.int32
    i16 = mybir.dt.int16
    Alu = mybir.AluOpType
    Act = mybir.ActivationFunctionType

    P = 128
    MT = m // P              # number of 128-row activation tiles
    KT = in_dim // P         # number of 128-deep contraction tiles
    FT = fp_dim // P         # outlier contraction tiles (2)
    OJ = os_dim // P         # out-shard subtiles (4)

    x = nc.dram_tensor("x", [m, in_dim], f32, kind="ExternalInput")
    qw = nc.dram_tensor("qw", [os_dim, in_dim // 2], i32, kind="ExternalInput")
    sc = nc.dram_tensor("sc", [os_dim], f32, kind="ExternalInput")
    wc = nc.dram_tensor("wc", [os_dim, fp_dim], f32, kind="ExternalInput")
    maskrow = nc.dram_tensor("maskrow", [in_dim], f32, kind="ExternalInput")
    idx = nc.dram_tensor("idx", [P, fp_dim // 16], i16, kind="ExternalInput")
    y = nc.dram_tensor("y", [m, os_dim], f32, kind="ExternalOutput")

    with (
        tc.tile_pool(name="const", bufs=1) as const,
        tc.tile_pool(name="wstage", bufs=1) as wstage,
        tc.tile_pool(name="xp", bufs=2) as xp,
        tc.tile_pool(name="xzp", bufs=2) as xzp,
        tc.tile_pool(name="qp", bufs=2) as qp,
        tc.tile_pool(name="qtp", bufs=2) as qtp,
        tc.tile_pool(name="aop", bufs=2) as aop,
        tc.tile_pool(name="aotp", bufs=2) as aotp,
        tc.tile_pool(name="sp", bufs=6) as sp,
        tc.tile_pool(name="yp", bufs=2) as yp,
        tc.tile_pool(name="py", bufs=2, space="PSUM") as py,
        tc.tile_pool(name="ptp", bufs=2, space="PSUM") as ptp,
    ):
        # ---------------- one-time setup ----------------
        from concourse import library_config

        if "gather" not in DBG:
            nc.gpsimd.load_library(library_config.ap_gather)

        identity = const.tile([P, P], f32)
        make_identity(nc, identity[:])

        # outlier mask broadcast to all partitions: maskF[p, k] = 0 iff k in ind
        maskF = const.tile([P, in_dim], f32)
        nc.sync.dma_start(maskF[:], maskrow[None, :].to_broadcast((P, in_dim)))

        # wrapped gather indices for ap_gather
        idxs = const.tile([P, fp_dim // 16], i16)
        nc.sync.dma_start(idxs[:], idx[:])

        # scale_col shard: broadcast along partitions [P, OS] for dequant
        sc_bcast = const.tile([P, os_dim], f32)
        nc.sync.dma_start(sc_bcast[:], sc[None, :].to_broadcast((P, os_dim)))

        # scale_col per-partition view [P, OJ] for pre-dividing weight_cache
        sc_op = const.tile([P, OJ], f32)
        nc.sync.dma_start(sc_op[:], sc.rearrange("(j p) -> p j", p=P))

        # weight_cache': wc[o, f] / sc[o], transposed to [P_f, FT, OS] bf16
        wc_sb = wstage.tile([P, OJ, fp_dim], f32)
        nc.sync.dma_start(wc_sb[:], wc.rearrange("(j p) f -> p j f", p=P))
        rsc_op = const.tile([P, OJ], f32)
        nc.vector.reciprocal(rsc_op[:], sc_op[:])
        wcp = wstage.tile([P, OJ, fp_dim], f32)
        for j in range(OJ):
            nc.vector.tensor_scalar(
                wcp[:, j, :], wc_sb[:, j, :], rsc_op[:, j : j + 1], None, Alu.mult
            )
        wcT = const.tile([P, FT, os_dim], f32r)
        for j in range(OJ):
            for ff in range(FT):
                ps = ptp.tile([P, P], f32, tag="tp")
                nc.tensor.transpose(ps[:], wcp[:, j, ff * P : (ff + 1) * P], identity[:])
                nc.scalar.activation(
                    wcT[:, ff, j * P : (j + 1) * P], ps[:], Act.Copy
                )

        # int4 weight unpack: qw[o, i] byte -> w[o, 2i] = lo nibble signed,
        # w[o, 2i+1] = hi nibble signed; then DMA-xbar into wT [P_k, KT, OS]
        wT = const.tile([P, KT, os_dim], bf16)
        qw_v = qw.rearrange("(j p) k -> p j k", p=P)
        for j in range(OJ):
            qwj = wstage.tile([P, in_dim // 2], i32, tag="qwj")
            nc.sync.dma_start(qwj[:], qw_v[:, j, :])
            w_ok = wstage.tile([P, in_dim], bf16, tag="wok")
            w_ok_v = w_ok.rearrange("p (k two) -> p k two", two=2)
            tmp = wstage.tile([P, in_dim // 2], i32, tag="wtmp")
            # low nibble: ((v & 15) ^ 8) - 8
            nc.vector.tensor_scalar(
                tmp[:], qwj[:], 15, 8, Alu.bitwise_and, Alu.bitwise_xor
            )
            nc.vector.tensor_scalar(w_ok_v[:, :, 0], tmp[:], 8, None, Alu.subtract)
            # high nibble: (((v >> 4) & 15) ^ 8) - 8
            tmp2 = wstage.tile([P, in_dim // 2], i32, tag="wtmp2")
            nc.vector.tensor_scalar(
                tmp2[:], qwj[:], 4, None, Alu.arith_shift_right
            )
            nc.vector.tensor_scalar(
                tmp[:], tmp2[:], 15, 8, Alu.bitwise_and, Alu.bitwise_xor
            )
            nc.vector.tensor_scalar(w_ok_v[:, :, 1], tmp[:], 8, None, Alu.subtract)
            # transpose [128 o, in_dim k] -> wT[p_k, KT, o-chunk j]
            nc.sync.dma_start_transpose(wT[:, :, j * P : (j + 1) * P], w_ok[:])

        # ---------------- main loop over 128-row activation tiles ----------
        for mi in range(MT):
            x_t = xp.tile([P, in_dim], f32)
            nc.sync.dma_start(x_t[:], x[mi * P : (mi + 1) * P, :])

            # masked abs-max -> mx; xz = x * mask (outlier cols zeroed)
            xz = xzp.tile([P, in_dim], f32)
            mx = sp.tile([P, 1], f32, tag="mx")
            nc.vector.tensor_tensor(xz[:], x_t[:], maskF[:], Alu.mult)
            nc.vector.tensor_reduce(
                mx[:], xz[:], mybir.AxisListType.X, Alu.max,
                apply_absolute_value=True,
            )
            s_t = sp.tile([P, 1], f32, tag="s")
            nc.vector.tensor_scalar(s_t[:], mx[:], float(np.float32(1.0) / np.float32(QMAX)), None, Alu.mult)
            r_t = sp.tile([P, 1], f32, tag="r")
            nc.vector.reciprocal(r_t[:], s_t[:])

            # outlier activations: gather + pre-scale by r, transpose via PE
            ao = aop.tile([P, fp_dim], f32, tag="ao")
            if "gather" in DBG:
                nc.vector.tensor_copy(ao[:], x_t[:, :fp_dim])
            else:
                nc.gpsimd.ap_gather(
                    ao[:, :, None],
                    x_t[:, :, None],
                    idxs[:],
                    channels=P,
                    num_elems=in_dim,
                    d=1,
                    num_idxs=fp_dim,
                )
            aos = aop.tile([P, fp_dim], f32, tag="aos")
            nc.vector.tensor_scalar(aos[:], ao[:], r_t[:], None, Alu.mult)
            aoT = aotp.tile([P, FT, P], f32r)
            for ff in range(FT):
                ps = ptp.tile([P, P], f32, tag="tp")
                nc.tensor.transpose(ps[:], aos[:, ff * P : (ff + 1) * P], identity[:])
                nc.scalar.activation(aoT[:, ff, :], ps[:], Act.Copy)

            # quantize: t = xz * r + MAGIC (ScalarE), q = t - MAGIC (GPSIMD)
            nc.scalar.activation(
                x_t[:], xz[:], Act.Copy, bias=MAGIC, scale=r_t[:]
            )
            q = qp.tile([P, in_dim], bf16)
            if "gpsimdq" in DBG:
                nc.vector.tensor_scalar(q[:], x_t[:], -MAGIC, None, Alu.add)
            else:
                nc.gpsimd.tensor_scalar(q[:], x_t[:], -MAGIC, None, Alu.add)

            # transpose q to contraction-major via DMA xbar
            qT = qtp.tile([P, KT, P], bf16)
            nc.sync.dma_start_transpose(qT[:], q[:])

            # GEMMs: 32 int tiles + 2 outlier tiles accumulate in one bank
            psum = py.tile([P, os_dim], f32)
            for ko in range(KT):
                nc.tensor.matmul(
                    psum[:],
                    qT[:, ko, :],
                    wT[:, ko, :],
                    start=(ko == 0),
                    stop=False,
                )
            for ff in range(FT):
                nc.tensor.matmul(
                    psum[:],
                    aoT[:, ff, :],
                    wcT[:, ff, :],
                    start=False,
                    stop=(ff == FT - 1),
                )

            # dequant + store: y = psum * x_scale (ACT) * scale_col (DVE)
            t1 = yp.tile([P, os_dim], f32, tag="t1")
            nc.scalar.activation(t1[:], psum[:], Act.Copy, scale=s_t[:])
            yt = yp.tile([P, os_dim], f32, tag="yt")
            nc.vector.scalar_tensor_tensor(
                yt[:], t1[:], 1.0, sc_bcast[:], Alu.mult, Alu.mult
            )
            nc.sync.dma_start(y[mi * P : (mi + 1) * P, :], yt[:])

    return nc


def build_nc(m=M, in_dim=IN, os_dim=OS, fp_dim=FP):
    import concourse.bacc as bacc
    import concourse.tile as tile

    nc = bacc.Bacc(None, target_bir_lowering=False)
    with tile.TileContext(nc) as tc:
        emit_core_kernel(nc, tc, m, in_dim, os_dim, fp_dim)
    nc.compile()
    return nc


def make_host_inputs(x, q_weight, scale_col, weight_cache, ind,
                     m=M, in_dim=IN, os_dim=OS, fp_dim=FP, ncores=NCORES):
    """Shard/relayout full inputs into per-core input maps (no arithmetic)."""
    xf = np.ascontiguousarray(x.reshape(m, in_dim).astype(np.float32, copy=False))
    ind = np.asarray(ind).astype(np.int64)
    maskrow = np.ones(in_dim, dtype=np.float32)
    maskrow[ind] = 0.0
    w = ind.astype(np.int16).reshape(fp_dim // 16, 16)  # j = i*16 + (p%16)
    idx = np.tile(w.T, (8, 1)).astype(np.int16)  # [128, fp/16]
    scf = np.asarray(scale_col).reshape(-1).astype(np.float32, copy=False)

    in_maps = []
    for c in range(ncores):
        o0, o1 = c * os_dim, (c + 1) * os_dim
        in_maps.append(
            {
                "x": xf,
                "qw": np.ascontiguousarray(q_weight[o0:o1]).astype(np.int32, copy=False),
                "sc": np.ascontiguousarray(scf[o0:o1]),
                "wc": np.ascontiguousarray(weight_cache[o0:o1]).astype(np.float32, copy=False),
                "maskrow": maskrow,
                "idx": idx,
            }
        )
    return in_maps


_NC_CACHE = {}


def kernel(x, q_weight, scale_col, weight_cache, ind, trace=False):
    from concourse.bass_utils import run_bass_kernel_spmd

    key = "full"
    if key not in _NC_CACHE:
        _NC_CACHE[key] = build_nc()
    nc = _NC_CACHE[key]

    in_maps = make_host_inputs(x, q_weight, scale_col, weight_cache, ind)
    res = run_bass_kernel_spmd(nc, in_maps, list(range(NCORES)), trace=trace)
    yshards = [res.results[c]["y"] for c in range(NCORES)]
    yfull = np.concatenate(yshards, axis=1).reshape(B, S, OUT)
    if trace:
        return yfull, res
    return yfull



# revision 29
# speedup vs baseline: 6.3568x; 6.3568x over previous
"""MixLinear int4-GEMM kernel for 8x TRN2 NeuronCores.

Strategy: 2-way M x 4-way OUT sharding.  Core c = mg*4 + og owns rows
[mg*4096, (mg+1)*4096) and output channels [og*1024, (og+1)*1024).  This
splits the per-row quantization work 4x vs pure OUT-sharding (which
duplicated it on all 8 cores) while keeping the whole bf16 weight shard
resident in SBUF.

Per core:
  Setup (once): int4 weight shard unpacked on DVE into bf16 and DMA-xbar
  transposed to contraction-major wT; outlier weight columns wc/sc
  (pre-divided by scale_col so one dequant covers everything) transposed
  via PE into wcT bf16.
  Per 128-row tile (32 tiles): masked abs-max -> x_scale; magic-number
  RNE round on ScalarE+DVE -> q (exact ints in bf16); DMA-xbar transpose
  to qT; GPSIMD outlier gather + ScalarE scale + PE transpose; 32+2 bf16
  matmuls into a [128, 1024] PSUM pair; dequant eviction; DMA out.

KERNEL_SAFE env (comma list) falls back to baseline-proven constructs:
  mask  - f32 mask, masked quantize (TT+reduce, no fused TTR / weight-col
          zeroing / in-place DVE)
  evict - two-step dequant (ScalarE scale, then DVE col-scale)
  qtp1  - single-buffered qT
"""

import os as _os

import numpy as np

B, S, IN, OUT, FP = 4, 2048, 4096, 4096, 256
M = B * S
NCORES = 8
MG, OG = 2, 4            # M-groups x OUT-groups
MS = M // MG             # rows per core (4096)
OS = OUT // OG           # out-features per core (1024)
QMAX = 7.0
MAGIC = 12582912.0       # 1.5 * 2**23: add+subtract forces RNE to integer


def emit_core_kernel(nc, tc, m, in_dim, os_dim, fp_dim):
    """Emit the per-core tile program. All dims compile-time constants."""
    import concourse.mybir as mybir
    from concourse import library_config
    from concourse.masks import make_identity

    SAFE = set(_os.environ.get("KERNEL_SAFE", "").split(","))

    f32 = mybir.dt.float32
    bf16 = mybir.dt.bfloat16
    i32 = mybir.dt.int32
    i16 = mybir.dt.int16
    Alu = mybir.AluOpType
    Act = mybir.ActivationFunctionType

    P = 128
    MT = m // P              # 32 activation row tiles
    KT = in_dim // P         # 32 contraction tiles
    FT = fp_dim // P         # 2 outlier contraction tiles
    OC = os_dim // P         # 8 out-shard 128-chunks
    NJ = os_dim // 512       # 2 psum chunks of 512

    x = nc.dram_tensor("x", [m, in_dim], f32, kind="ExternalInput")
    qw = nc.dram_tensor("qw", [os_dim, in_dim // 2], i32, kind="ExternalInput")
    sc = nc.dram_tensor("sc", [os_dim], f32, kind="ExternalInput")
    wc = nc.dram_tensor("wc", [os_dim, fp_dim], f32, kind="ExternalInput")
    maskrow = nc.dram_tensor("maskrow", [in_dim], f32, kind="ExternalInput")
    idx = nc.dram_tensor("idx", [P, fp_dim // 16], i16, kind="ExternalInput")
    y = nc.dram_tensor("y", [m, os_dim], f32, kind="ExternalOutput")

    with (
        tc.tile_pool(name="const", bufs=1) as const,
        tc.tile_pool(name="wstage", bufs=1) as wstage,
        tc.tile_pool(name="xp", bufs=2) as xp,
        tc.tile_pool(name="xzp", bufs=1) as xzp,
        tc.tile_pool(name="qp", bufs=2) as qp,
        tc.tile_pool(name="qtp", bufs=1 if "qtp1" in SAFE else 2) as qtp,
        tc.tile_pool(name="aop", bufs=2) as aop,
        tc.tile_pool(name="sp", bufs=8) as sp,
        tc.tile_pool(name="yp", bufs=1 if "evict" in SAFE else 2) as yp,
        tc.tile_pool(name="py", bufs=2, space="PSUM") as py,
        tc.tile_pool(name="ptp", bufs=2, space="PSUM") as ptp,
    ):
        # ---------------- one-time setup ----------------
        nc.gpsimd.load_library(library_config.ap_gather)

        identity = const.tile([P, P], f32)
        make_identity(nc, identity[:])

        # outlier mask broadcast to all partitions, bf16 (0/1 exact;
        # halves the mask read bandwidth in the masking TT pass)
        maskF = const.tile([P, in_dim], bf16, name="maskF")
        mtmp = xzp.tile([P, in_dim], f32, tag="xz")
        nc.sync.dma_start(mtmp[:], maskrow[None, :].to_broadcast((P, in_dim)))
        nc.scalar.activation(maskF[:], mtmp[:], Act.Copy)

        # wrapped gather indices for ap_gather
        idxs = const.tile([P, fp_dim // 16], i16)
        nc.sync.dma_start(idxs[:], idx[:])

        # scale_col shard broadcast along partitions [P, OS] for dequant
        sc_bcast = const.tile([P, os_dim], f32)
        nc.sync.dma_start(sc_bcast[:], sc[None, :].to_broadcast((P, os_dim)))

        # scale_col per-partition view [P, OC] for pre-dividing weight_cache
        sc_op = const.tile([P, OC], f32)
        nc.sync.dma_start(sc_op[:], sc.rearrange("(c p) -> p c", p=P))
        rsc_op = const.tile([P, OC], f32)
        nc.vector.reciprocal(rsc_op[:], sc_op[:])

        # int4 weight unpack into contraction-major bf16 wT (per j-half so
        # early matmuls only wait on their own half), and weight_cache
        # pre-scaled by 1/scale_col into wcT.
        wT = [
            const.tile([P, KT, 512], bf16, name=f"wT{j}", tag=f"wT{j}")
            for j in range(NJ)
        ]
        wcT = const.tile([P, FT, os_dim], bf16)
        qw_v = qw.rearrange("(c p) k -> p c k", p=P)
        wc_v = wc.rearrange("(c p) f -> p c f", p=P)
        for c in range(OC):
            qwj = wstage.tile([P, in_dim // 2], i32, tag="qwj", bufs=2)
            nc.sync.dma_start(qwj[:], qw_v[:, c, :])
            w_ok = wstage.tile([P, in_dim], bf16, tag="wok")
            w_ok_v = w_ok.rearrange("p (k two) -> p k two", two=2)
            tmp = wstage.tile([P, in_dim // 2], i32, tag="wtmp")
            # low nibble: ((v & 15) ^ 8) - 8
            nc.vector.tensor_scalar(
                tmp[:], qwj[:], 15, 8, Alu.bitwise_and, Alu.bitwise_xor
            )
            nc.vector.tensor_scalar(w_ok_v[:, :, 0], tmp[:], 8, None, Alu.subtract)
            # high nibble: same decode after v >>= 4 (ping-pong, no in-place)
            nc.vector.tensor_scalar(tmp[:], qwj[:], 4, None, Alu.arith_shift_right)
            nc.vector.tensor_scalar(
                qwj[:], tmp[:], 15, 8, Alu.bitwise_and, Alu.bitwise_xor
            )
            nc.vector.tensor_scalar(w_ok_v[:, :, 1], qwj[:], 8, None, Alu.subtract)
            # transpose [128 o, in_dim k] -> wT[j][p_k, KT, 128-chunk]
            j, cc = c // (OC // NJ), c % (OC // NJ)
            nc.sync.dma_start_transpose(
                wT[j][:, :, cc * P : (cc + 1) * P], w_ok[:]
            )

            # outlier weights: wcp = wc[o, f] / sc[o], PE transpose (f32
            # PSUM; converted to bf16 at the ScalarE eviction)
            wcc = wstage.tile([P, fp_dim], f32, tag="wcc")
            nc.sync.dma_start(wcc[:], wc_v[:, c, :])
            wcp = wstage.tile([P, fp_dim], f32, tag="wcp")
            nc.scalar.activation(
                wcp[:], wcc[:], Act.Copy, scale=rsc_op[:, c : c + 1]
            )
            for ff in range(FT):
                ps = ptp.tile([P, P], f32, tag="tp")
                nc.tensor.transpose(
                    ps[:], wcp[:, ff * P : (ff + 1) * P], identity[:]
                )
                nc.scalar.activation(
                    wcT[:, ff, c * P : (c + 1) * P], ps[:], Act.Copy
                )

        # ---------------- main loop over 128-row activation tiles ----------
        inv7 = float(np.float32(1.0) / np.float32(QMAX))
        for mi in range(MT):
            x_t = xp.tile([P, in_dim], f32)
            nc.sync.dma_start(x_t[:], x[mi * P : (mi + 1) * P, :])

            # outlier activations (full precision, pre-masking)
            ao = aop.tile([P, fp_dim], f32, tag="ao")
            nc.gpsimd.ap_gather(
                ao[:, :, None],
                x_t[:, :, None],
                idxs[:],
                channels=P,
                num_elems=in_dim,
                d=1,
                num_idxs=fp_dim,
            )

            # xz = x*mask (one DVE pass, bf16 mask); mx = absmax(xz).
            # NOTE: the fused tensor_tensor_reduce hangs real TRN2 hardware
            # (passes CoreSim) -- keep the two-instruction form.
            axs = xzp.tile([P, in_dim], f32, tag="xz")
            mx = sp.tile([P, 1], f32, tag="mx")
            nc.vector.tensor_tensor(axs[:], x_t[:], maskF[:], Alu.mult)
            nc.vector.tensor_reduce(
                mx[:], axs[:], mybir.AxisListType.X, Alu.max,
                apply_absolute_value=True,
            )
            qsrc = axs
            s_t = sp.tile([P, 1], f32, tag="s")
            nc.vector.tensor_scalar(s_t[:], mx[:], inv7, None, Alu.mult)
            r_t = sp.tile([P, 1], f32, tag="r")
            nc.vector.reciprocal(r_t[:], s_t[:])

            # outliers scaled by r (ScalarE; per-partition scale), PE transpose
            aos = aop.tile([P, fp_dim], f32, tag="aos")
            nc.scalar.activation(aos[:], ao[:], Act.Copy, scale=r_t[:])
            aoT = aop.tile([P, FT, P], bf16, tag="aoT", bufs=1)
            for ff in range(FT):
                ps = ptp.tile([P, P], f32, tag="tp")
                nc.tensor.transpose(
                    ps[:], aos[:, ff * P : (ff + 1) * P], identity[:]
                )
                nc.scalar.activation(aoT[:, ff, :], ps[:], Act.Copy)

            # quantize: t = qsrc * r + MAGIC (ScalarE), q = t - MAGIC (DVE 2x).
            # t lands in whichever of x_t/axs is no longer needed.
            tq = x_t if qsrc is axs else axs
            nc.scalar.activation(tq[:], qsrc[:], Act.Copy, bias=MAGIC, scale=r_t[:])
            q = qp.tile([P, in_dim], bf16, tag="q")
            nc.vector.tensor_scalar(q[:], tq[:], -MAGIC, None, Alu.add)

            # transpose q to contraction-major via DMA xbar
            qT = qtp.tile([P, KT, P], bf16)
            nc.sync.dma_start_transpose(qT[:], q[:])

            # GEMMs: (32 int + 2 outlier) matmuls x NJ psum chunks of 512
            psum = py.tile([P, os_dim], f32)
            for ko in range(KT):
                for j in range(NJ):
                    nc.tensor.matmul(
                        psum[:, j * 512 : (j + 1) * 512],
                        qT[:, ko, :],
                        wT[j][:, ko, :],
                        start=(ko == 0),
                        stop=False,
                    )
            for ff in range(FT):
                for j in range(NJ):
                    nc.tensor.matmul(
                        psum[:, j * 512 : (j + 1) * 512],
                        aoT[:, ff, :],
                        wcT[:, ff, j * 512 : (j + 1) * 512],
                        start=False,
                        stop=(ff == FT - 1),
                    )

            # dequant + store: y = (psum * x_scale) * scale_col
            yt = yp.tile([P, os_dim], f32, tag="yt")
            if "evict" in SAFE:
                t1 = yp.tile([P, os_dim], f32, tag="t1")
                nc.scalar.activation(t1[:], psum[:], Act.Copy, scale=s_t[:])
                nc.vector.scalar_tensor_tensor(
                    yt[:], t1[:], 1.0, sc_bcast[:], Alu.mult, Alu.mult
                )
            else:
                nc.vector.scalar_tensor_tensor(
                    yt[:], psum[:], s_t[:], sc_bcast[:], Alu.mult, Alu.mult
                )
            nc.sync.dma_start(y[mi * P : (mi + 1) * P, :], yt[:])

    return nc


def build_nc(m=MS, in_dim=IN, os_dim=OS, fp_dim=FP):
    import concourse.bacc as bacc
    import concourse.tile as tile

    nc = bacc.Bacc(None, target_bir_lowering=False)
    with tile.TileContext(nc) as tc:
        emit_core_kernel(nc, tc, m, in_dim, os_dim, fp_dim)
    nc.compile()
    return nc


def make_host_inputs(x, q_weight, scale_col, weight_cache, ind,
                     in_dim=IN, os_dim=OS, ms=MS, fp_dim=FP):
    """Shard/relayout full inputs into per-core input maps (no arithmetic)."""
    xf = np.ascontiguousarray(x.reshape(M, in_dim).astype(np.float32, copy=False))
    ind = np.asarray(ind).astype(np.int64)
    maskrow = np.ones(in_dim, dtype=np.float32)
    maskrow[ind] = 0.0
    w = ind.astype(np.int16).reshape(fp_dim // 16, 16)  # j = i*16 + (p%16)
    idx = np.tile(w.T, (8, 1)).astype(np.int16)  # [128, fp/16]
    scf = np.asarray(scale_col).reshape(-1).astype(np.float32, copy=False)

    in_maps = []
    for c in range(NCORES):
        mg, og = c // OG, c % OG
        o0, o1 = og * os_dim, (og + 1) * os_dim
        in_maps.append(
            {
                "x": xf[mg * ms : (mg + 1) * ms],
                "qw": np.ascontiguousarray(q_weight[o0:o1]).astype(np.int32, copy=False),
                "sc": np.ascontiguousarray(scf[o0:o1]),
                "wc": np.ascontiguousarray(weight_cache[o0:o1]).astype(np.float32, copy=False),
                "maskrow": maskrow,
                "idx": idx,
            }
        )
    return in_maps


_NC_CACHE = {}


def kernel(x, q_weight, scale_col, weight_cache, ind, trace=False):
    from concourse.bass_utils import run_bass_kernel_spmd

    key = "full"
    if key not in _NC_CACHE:
        _NC_CACHE[key] = build_nc()
    nc = _NC_CACHE[key]

    in_maps = make_host_inputs(x, q_weight, scale_col, weight_cache, ind)
    res = run_bass_kernel_spmd(nc, in_maps, list(range(NCORES)), trace=trace)
    yfull = np.empty((M, OUT), dtype=np.float32)
    for c in range(NCORES):
        mg, og = c // OG, c % OG
        yfull[mg * MS : (mg + 1) * MS, og * OS : (og + 1) * OS] = res.results[c]["y"]
    yfull = yfull.reshape(B, S, OUT)
    if trace:
        return yfull, res
    return yfull


# revision 31
# speedup vs baseline: 6.5021x; 1.0229x over previous
"""MixLinear int4-GEMM kernel for 8x TRN2 NeuronCores.

Strategy: 2-way M x 4-way OUT sharding.  Core c = mg*4 + og owns rows
[mg*4096, (mg+1)*4096) and output channels [og*1024, (og+1)*1024).  This
splits the per-row quantization work 4x vs pure OUT-sharding (which
duplicated it on all 8 cores) while keeping the whole bf16 weight shard
resident in SBUF.

Per core:
  Setup (once): int4 weight shard unpacked on DVE into bf16 and DMA-xbar
  transposed to contraction-major wT; outlier weight columns wc/sc
  (pre-divided by scale_col so one dequant covers everything) transposed
  via PE into wcT bf16.
  Per 128-row tile (32 tiles): masked abs-max -> x_scale; magic-number
  RNE round on ScalarE+DVE -> q (exact ints in bf16); DMA-xbar transpose
  to qT; GPSIMD outlier gather + ScalarE scale + PE transpose; 32+2 bf16
  matmuls into a [128, 1024] PSUM pair; dequant eviction; DMA out.

KERNEL_SAFE env (comma list) falls back to baseline-proven constructs:
  mask  - f32 mask, masked quantize (TT+reduce, no fused TTR / weight-col
          zeroing / in-place DVE)
  evict - two-step dequant (ScalarE scale, then DVE col-scale)
  qtp1  - single-buffered qT
"""

import os as _os

import numpy as np

B, S, IN, OUT, FP = 4, 2048, 4096, 4096, 256
M = B * S
NCORES = 8
MG, OG = 2, 4            # M-groups x OUT-groups
MS = M // MG             # rows per core (4096)
OS = OUT // OG           # out-features per core (1024)
QMAX = 7.0
MAGIC = 12582912.0       # 1.5 * 2**23: add+subtract forces RNE to integer


def emit_core_kernel(nc, tc, m, in_dim, os_dim, fp_dim):
    """Emit the per-core tile program. All dims compile-time constants."""
    import concourse.mybir as mybir
    from concourse import library_config
    from concourse.masks import make_identity

    SAFE = set(_os.environ.get("KERNEL_SAFE", "").split(","))

    f32 = mybir.dt.float32
    bf16 = mybir.dt.bfloat16
    i32 = mybir.dt.int32
    i16 = mybir.dt.int16
    Alu = mybir.AluOpType
    Act = mybir.ActivationFunctionType

    P = 128
    MT = m // P              # 32 activation row tiles
    KT = in_dim // P         # 32 contraction tiles
    FT = fp_dim // P         # 2 outlier contraction tiles
    OC = os_dim // P         # 8 out-shard 128-chunks
    NJ = os_dim // 512       # 2 psum chunks of 512

    x = nc.dram_tensor("x", [m, in_dim], f32, kind="ExternalInput")
    qw = nc.dram_tensor("qw", [os_dim, in_dim // 2], i32, kind="ExternalInput")
    sc = nc.dram_tensor("sc", [os_dim], f32, kind="ExternalInput")
    wc = nc.dram_tensor("wc", [os_dim, fp_dim], f32, kind="ExternalInput")
    maskrow = nc.dram_tensor("maskrow", [in_dim], f32, kind="ExternalInput")
    idx = nc.dram_tensor("idx", [P, fp_dim // 16], i16, kind="ExternalInput")
    y = nc.dram_tensor("y", [m, os_dim], f32, kind="ExternalOutput")

    with (
        tc.tile_pool(name="const", bufs=1) as const,
        tc.tile_pool(name="wstage", bufs=1) as wstage,
        tc.tile_pool(name="xp", bufs=2) as xp,
        tc.tile_pool(name="xzp", bufs=1) as xzp,
        tc.tile_pool(name="qp", bufs=2) as qp,
        tc.tile_pool(name="qtp", bufs=1 if "qtp1" in SAFE else 2) as qtp,
        tc.tile_pool(name="aop", bufs=2) as aop,
        tc.tile_pool(name="sp", bufs=8) as sp,
        tc.tile_pool(name="yp", bufs=1 if "evict" in SAFE else 2) as yp,
        tc.tile_pool(name="py", bufs=3, space="PSUM") as py,
        tc.tile_pool(name="ptp", bufs=2, space="PSUM") as ptp,
    ):
        # ---------------- one-time setup ----------------
        nc.gpsimd.load_library(library_config.ap_gather)

        identity = const.tile([P, P], f32)
        make_identity(nc, identity[:])

        # outlier mask broadcast to all partitions, bf16 (0/1 exact;
        # halves the mask read bandwidth in the masking TT pass)
        maskF = const.tile([P, in_dim], bf16, name="maskF")
        mtmp = xzp.tile([P, in_dim], f32, tag="xz")
        nc.sync.dma_start(mtmp[:], maskrow[None, :].to_broadcast((P, in_dim)))
        nc.scalar.activation(maskF[:], mtmp[:], Act.Copy)

        # wrapped gather indices for ap_gather
        idxs = const.tile([P, fp_dim // 16], i16)
        nc.sync.dma_start(idxs[:], idx[:])

        # scale_col shard broadcast along partitions [P, OS] for dequant
        sc_bcast = const.tile([P, os_dim], f32)
        nc.sync.dma_start(sc_bcast[:], sc[None, :].to_broadcast((P, os_dim)))

        # scale_col per-partition view [P, OC] for pre-dividing weight_cache
        sc_op = const.tile([P, OC], f32)
        nc.sync.dma_start(sc_op[:], sc.rearrange("(c p) -> p c", p=P))
        rsc_op = const.tile([P, OC], f32)
        nc.vector.reciprocal(rsc_op[:], sc_op[:])

        # int4 weight unpack into contraction-major bf16 wT (per j-half so
        # early matmuls only wait on their own half), and weight_cache
        # pre-scaled by 1/scale_col into wcT.
        wT = [
            const.tile([P, KT, 512], bf16, name=f"wT{j}", tag=f"wT{j}")
            for j in range(NJ)
        ]
        wcT = const.tile([P, FT, os_dim], bf16)
        qw_v = qw.rearrange("(c p) k -> p c k", p=P)
        wc_v = wc.rearrange("(c p) f -> p c f", p=P)
        for c in range(OC):
            qwj = wstage.tile([P, in_dim // 2], i32, tag="qwj", bufs=2)
            nc.sync.dma_start(qwj[:], qw_v[:, c, :])
            w_ok = wstage.tile([P, in_dim], bf16, tag="wok")
            w_ok_v = w_ok.rearrange("p (k two) -> p k two", two=2)
            tmp = wstage.tile([P, in_dim // 2], i32, tag="wtmp")
            # low nibble: ((v & 15) ^ 8) - 8
            nc.vector.tensor_scalar(
                tmp[:], qwj[:], 15, 8, Alu.bitwise_and, Alu.bitwise_xor
            )
            nc.vector.tensor_scalar(w_ok_v[:, :, 0], tmp[:], 8, None, Alu.subtract)
            # high nibble: same decode after v >>= 4 (ping-pong, no in-place)
            nc.vector.tensor_scalar(tmp[:], qwj[:], 4, None, Alu.arith_shift_right)
            nc.vector.tensor_scalar(
                qwj[:], tmp[:], 15, 8, Alu.bitwise_and, Alu.bitwise_xor
            )
            nc.vector.tensor_scalar(w_ok_v[:, :, 1], qwj[:], 8, None, Alu.subtract)
            # transpose [128 o, in_dim k] -> wT[j][p_k, KT, 128-chunk]
            j, cc = c // (OC // NJ), c % (OC // NJ)
            nc.sync.dma_start_transpose(
                wT[j][:, :, cc * P : (cc + 1) * P], w_ok[:]
            )

            # outlier weights: wcp = wc[o, f] / sc[o], PE transpose (f32
            # PSUM; converted to bf16 at the ScalarE eviction)
            wcc = wstage.tile([P, fp_dim], f32, tag="wcc")
            nc.sync.dma_start(wcc[:], wc_v[:, c, :])
            wcp = wstage.tile([P, fp_dim], f32, tag="wcp")
            nc.scalar.activation(
                wcp[:], wcc[:], Act.Copy, scale=rsc_op[:, c : c + 1]
            )
            for ff in range(FT):
                ps = ptp.tile([P, P], f32, tag="tp")
                nc.tensor.transpose(
                    ps[:], wcp[:, ff * P : (ff + 1) * P], identity[:]
                )
                nc.scalar.activation(
                    wcT[:, ff, c * P : (c + 1) * P], ps[:], Act.Copy
                )

        # ---------------- main loop over 128-row activation tiles ----------
        inv7 = float(np.float32(1.0) / np.float32(QMAX))
        for mi in range(MT):
            x_t = xp.tile([P, in_dim], f32)
            nc.sync.dma_start(x_t[:], x[mi * P : (mi + 1) * P, :])

            # outlier activations (full precision, pre-masking)
            ao = aop.tile([P, fp_dim], f32, tag="ao")
            nc.gpsimd.ap_gather(
                ao[:, :, None],
                x_t[:, :, None],
                idxs[:],
                channels=P,
                num_elems=in_dim,
                d=1,
                num_idxs=fp_dim,
            )

            # xz = x*mask (one DVE pass, bf16 mask); mx = absmax(xz).
            # NOTE: the fused tensor_tensor_reduce hangs real TRN2 hardware
            # (passes CoreSim) -- keep the two-instruction form.
            axs = xzp.tile([P, in_dim], f32, tag="xz")
            mx = sp.tile([P, 1], f32, tag="mx")
            nc.vector.tensor_tensor(axs[:], x_t[:], maskF[:], Alu.mult)
            nc.vector.tensor_reduce(
                mx[:], axs[:], mybir.AxisListType.X, Alu.max,
                apply_absolute_value=True,
            )
            qsrc = axs
            s_t = sp.tile([P, 1], f32, tag="s")
            nc.vector.tensor_scalar(s_t[:], mx[:], inv7, None, Alu.mult)
            r_t = sp.tile([P, 1], f32, tag="r")
            nc.vector.reciprocal(r_t[:], s_t[:])

            # outliers scaled by r (ScalarE; per-partition scale), PE transpose
            aos = aop.tile([P, fp_dim], f32, tag="aos")
            nc.scalar.activation(aos[:], ao[:], Act.Copy, scale=r_t[:])
            aoT = aop.tile([P, FT, P], bf16, tag="aoT", bufs=1)
            for ff in range(FT):
                ps = ptp.tile([P, P], f32, tag="tp")
                nc.tensor.transpose(
                    ps[:], aos[:, ff * P : (ff + 1) * P], identity[:]
                )
                nc.scalar.activation(aoT[:, ff, :], ps[:], Act.Copy)

            # quantize: t = qsrc * r + MAGIC (ScalarE), q = t - MAGIC (DVE 2x).
            # t lands in whichever of x_t/axs is no longer needed.
            tq = x_t if qsrc is axs else axs
            nc.scalar.activation(tq[:], qsrc[:], Act.Copy, bias=MAGIC, scale=r_t[:])
            q = qp.tile([P, in_dim], bf16, tag="q")
            nc.vector.tensor_scalar(q[:], tq[:], -MAGIC, None, Alu.add)

            # transpose q to contraction-major via DMA xbar
            qT = qtp.tile([P, KT, P], bf16)
            nc.sync.dma_start_transpose(qT[:], q[:])

            # GEMMs: (32 int + 2 outlier) matmuls per psum chunk
            psum = py.tile([P, os_dim], f32)
            if mi < 4:
                # ramp tiles: N=128 per weight 128-chunk, emitted chunk-major
                # so each chunk's matmuls start as soon as ITS unpack lands
                # (N=512 needs 4 chunks; j-interleaved needs all 8)
                for c in range(OC):
                    j, cc = c // (OC // NJ), c % (OC // NJ)
                    pslice = psum[:, c * P : (c + 1) * P]
                    for ko in range(KT):
                        nc.tensor.matmul(
                            pslice,
                            qT[:, ko, :],
                            wT[j][:, ko, cc * P : (cc + 1) * P],
                            start=(ko == 0),
                            stop=False,
                        )
                    for ff in range(FT):
                        nc.tensor.matmul(
                            pslice,
                            aoT[:, ff, :],
                            wcT[:, ff, c * P : (c + 1) * P],
                            start=False,
                            stop=(ff == FT - 1),
                        )
            else:
                for ko in range(KT):
                    for j in range(NJ):
                        nc.tensor.matmul(
                            psum[:, j * 512 : (j + 1) * 512],
                            qT[:, ko, :],
                            wT[j][:, ko, :],
                            start=(ko == 0),
                            stop=False,
                        )
                for ff in range(FT):
                    for j in range(NJ):
                        nc.tensor.matmul(
                            psum[:, j * 512 : (j + 1) * 512],
                            aoT[:, ff, :],
                            wcT[:, ff, j * 512 : (j + 1) * 512],
                            start=False,
                            stop=(ff == FT - 1),
                        )

            # dequant + store: y = (psum * x_scale) * scale_col
            yt = yp.tile([P, os_dim], f32, tag="yt")
            if "evict" in SAFE:
                t1 = yp.tile([P, os_dim], f32, tag="t1")
                nc.scalar.activation(t1[:], psum[:], Act.Copy, scale=s_t[:])
                nc.vector.scalar_tensor_tensor(
                    yt[:], t1[:], 1.0, sc_bcast[:], Alu.mult, Alu.mult
                )
            else:
                nc.vector.scalar_tensor_tensor(
                    yt[:], psum[:], s_t[:], sc_bcast[:], Alu.mult, Alu.mult
                )
            nc.sync.dma_start(y[mi * P : (mi + 1) * P, :], yt[:])

    return nc


def build_nc(m=MS, in_dim=IN, os_dim=OS, fp_dim=FP):
    import concourse.bacc as bacc
    import concourse.tile as tile

    nc = bacc.Bacc(None, target_bir_lowering=False)
    with tile.TileContext(nc) as tc:
        emit_core_kernel(nc, tc, m, in_dim, os_dim, fp_dim)
    nc.compile()
    return nc


def make_host_inputs(x, q_weight, scale_col, weight_cache, ind,
                     in_dim=IN, os_dim=OS, ms=MS, fp_dim=FP):
    """Shard/relayout full inputs into per-core input maps (no arithmetic)."""
    xf = np.ascontiguousarray(x.reshape(M, in_dim).astype(np.float32, copy=False))
    ind = np.asarray(ind).astype(np.int64)
    maskrow = np.ones(in_dim, dtype=np.float32)
    maskrow[ind] = 0.0
    w = ind.astype(np.int16).reshape(fp_dim // 16, 16)  # j = i*16 + (p%16)
    idx = np.tile(w.T, (8, 1)).astype(np.int16)  # [128, fp/16]
    scf = np.asarray(scale_col).reshape(-1).astype(np.float32, copy=False)

    in_maps = []
    for c in range(NCORES):
        mg, og = c // OG, c % OG
        o0, o1 = og * os_dim, (og + 1) * os_dim
        in_maps.append(
            {
                "x": xf[mg * ms : (mg + 1) * ms],
                "qw": np.ascontiguousarray(q_weight[o0:o1]).astype(np.int32, copy=False),
                "sc": np.ascontiguousarray(scf[o0:o1]),
                "wc": np.ascontiguousarray(weight_cache[o0:o1]).astype(np.float32, copy=False),
                "maskrow": maskrow,
                "idx": idx,
            }
        )
    return in_maps


_NC_CACHE = {}


def kernel(x, q_weight, scale_col, weight_cache, ind, trace=False):
    from concourse.bass_utils import run_bass_kernel_spmd

    key = "full"
    if key not in _NC_CACHE:
        _NC_CACHE[key] = build_nc()
    nc = _NC_CACHE[key]

    in_maps = make_host_inputs(x, q_weight, scale_col, weight_cache, ind)
    res = run_bass_kernel_spmd(nc, in_maps, list(range(NCORES)), trace=trace)
    yfull = np.empty((M, OUT), dtype=np.float32)
    for c in range(NCORES):
        mg, og = c // OG, c % OG
        yfull[mg * MS : (mg + 1) * MS, og * OS : (og + 1) * OS] = res.results[c]["y"]
    yfull = yfull.reshape(B, S, OUT)
    if trace:
        return yfull, res
    return yfull
